# revision 49
# baseline (speedup 1.0000x reference)
"""Trainium2 Bass kernel for nn_MultiHeadAttention_53463752900838.

Math (per batch element b, one NeuronCore each — pure data parallel over B=8):
  qkv = w_qkv @ x + b_qkv                     (3072, T)
  q,k,v per head h: (64, T)
  scores[t,h,g] = sum_d (q[h,d,t]/8) k[g,d,t] per-timestep 16x16 Gram matrix
  attn = softmax over t  (per (h,g) pair)
  context[h,d,t] = sum_g attn[t,h,g] v[g,d,t]
  out = w_out @ context + b_out               (1024, T)

Cost-model-driven layout (fp8e4 DoubleRow matmuls where they pay, bf16 rest):

  Pass 1 (per 512-t span, m-tile order K,Q,V): QKV projection in fp8
    DoubleRow, two accumulation sets per m-tile: W32@x8 (weights x32 on
    host) plus the x-residual correction W32@xr with xr = fp8(x - fp8(x)),
    which recovers near-bf16 accuracy on the x operand at half the bf16
    matmul cost (4+4 DR matmuls of paired c-chunks into a (128,512) PSUM
    bank; optional third set W_r@x8 under USE_WR). Evacuations add the x32
    bias (Act activation / DVE tensor_scalar, weighted round-robin); Q/K
    land directly in the scores layout (64d, (head,t)) bf16 via strided
    half-height writes; V is staged and spilled to DRAM as (g,(d,t)).
    Per-t 16x16 bf16 Gram matmuls (the /8 and 32^2 scales fold into the
    Act exp's 1/8192) interleave one 32-t block per q/k m-tile; the last
    span interleaves its own scores over its V m-tiles. Z bookkeeping is
    quartered: spans 0..6 fold into a bf16 sesum tree (DVE 2x), the big
    fp32 reduce runs quartered under the last span, whose quarters reduce
    straight into zmain, so only recip + a (16,(16h,64)) broadcast of
    S_A/Z sit on the tail.

  Pass 2 (per 256-t sub-span): attn = se * (S_A/Z) in quartered DVE 2x
    tensor_tensors against the pre-broadcast reciprocal tile. Context
    matmuls write PSUM directly in out-projection layout: per t two
    matmuls (even/odd heads hl) put (64d x 8he) at partition offset 64*hl
    with he strided by 64 columns, so each (128,(8he,64t)) PSUM bank
    evacuates 1:1 into the fp8 cnat tile ((hl,d) x (he,t)) with zero
    marshal DMAs. Out-projection is fp8 DoubleRow over 4 he-chunk pairs
    (the (he,hl,d) channel order equals h*64+d, so w_out needs no
    permutation), interleaved per 128-t tile between context quarters;
    output (t,o) bf16 stored with scale 1/(32*32*S_A); the host transposes
    and adds b_out in fp32.

  Measured on hardware: rel err 1.8298e-2 (gate 2e-2, deterministic);
  TimelineSim 298.1us vs 654.9us baseline.
"""

import os
import sys
import contextlib

import numpy as np
import ml_dtypes

for p in ("/opt/trn_rl_repo",):
    if p not in sys.path and os.path.isdir(p):
        sys.path.insert(0, p)

import concourse.bass as bass
import concourse.tile as tile
from concourse import mybir
from concourse.bass_utils import run_bass_kernel_spmd

F32 = mybir.dt.float32
BF16 = mybir.dt.bfloat16
FP8 = mybir.dt.float8e4

N_CORES = 8
C = 1024
H = 16
DK = 64
OC3 = 3072
S_A = 16.0  # attn scale so fp8 context is well-conditioned (32*16*ctx < 240)
W_SCALE = 32.0  # host-side weight scale into fp8 range
# fp8 residual-correction sets: "xr" (x residual, all of qkv) is required to
# meet the 2e-2 gate; "wr" (weight residual on q/k) adds ~1.3x extra margin.
# Host-sim (bit-faithful, verified == HW): xr only -> 1.7786e-2; +wr -> 1.5569e-2.
USE_WR = False

_WAITS2_OK = {
    "InstMatmult",
    "InstLdweights",
    "InstTensorCopy",
    "InstActivation",
    "InstTensorTensor",
    "InstTensorReduce",
    "InstDMACopy",
    "InstTensorScalarPtr",
    "InstMemset",
}


def _dedup_ldweights(nc):
    """Drop sync-free InstLdweights whose weights AP matches the weights
    already in the PE array (loaded by the previous Ldweights or self-loading
    Matmult). The Matmult keeps its own [ifmap, weights] operands, so codegen
    is unaffected; this removes redundant PE sequencer work."""
    n = 0
    for fn in nc.m.functions:
        for bb in fn.blocks:
            out = []
            sig = None
            for ins in bb.instructions:
                eng = str(ins.engine)
                if not eng.endswith("PE"):
                    out.append(ins)
                    continue
                op = ins.opcode
                if op == "Ldweights":
                    si = getattr(ins, "sync_info", None)
                    free = si is None or (not si.on_wait and not si.on_update)
                    s = repr(ins.ins[0]) if ins.ins else None
                    if free and s is not None and s == sig:
                        n += 1
                        continue  # drop
                    sig = s
                elif op == "Matmult":
                    sig = repr(ins.ins[1]) if len(ins.ins) > 1 else None
                elif op in ("NoOp",):
                    pass
                else:
                    sig = None
                out.append(ins)
            bb.instructions[:] = out
    return n


def _split_sync_waits(nc, limit=1):
    """walrus codegen rejects too many semaphore waits per instruction (CTRL
    class takes 1); hoist overflow waits onto NoOps inserted before the
    offending instruction."""
    counter = [0]
    n_split = 0
    for fn in nc.m.functions:
        for bb in fn.blocks:
            out = []
            for ins in bb.instructions:
                si = getattr(ins, "sync_info", None)
                waits = list(si.on_wait) if (si is not None and si.on_wait) else []
                if len(waits) > limit:
                    n_split += 1
                    extra, keep = waits[:-limit], waits[-limit:]
                    for i in range(0, len(extra), limit):
                        counter[0] += 1
                        out.append(
                            mybir.InstNoOp(
                                name=f"I-wsplit-{counter[0]}",
                                opcode="NoOp",
                                engine=ins.engine,
                                ins=[],
                                outs=[],
                                sync_info=mybir.SyncInfo(
                                    on_wait=list(extra[i : i + limit]), on_update=[]
                                ),
                            )
                        )
                    si.on_wait = keep
                out.append(ins)
            bb.instructions[:] = out
    return n_split


class _EvacPicker:
    """Weighted rotation over (engine, kind) for PSUM evacuations."""

    def __init__(self, nc, weights):
        # weights: list of (name, weight); name in {"act", "dve", "pool"}
        self.nc = nc
        self.entries = [[name, float(w), 0.0] for name, w in weights]
        self.total = sum(w for _, w in weights)

    def pick(self):
        for e in self.entries:
            e[2] += e[1]
        best = max(self.entries, key=lambda e: e[2])
        best[2] -= self.total
        return best[0]


def build_kernel(T=4096, SPAN=256):
    SP1 = 512  # pass-1 span
    NSP1 = T // SP1
    SS = 256  # pass-2 sub-span
    NSS = T // SS
    nc = bass.Bass("TRN2", target_bir_lowering=False, debug=False)

    x_in = nc.dram_tensor("x", [C, T], FP8, kind="ExternalInput").ap()
    xr_in = nc.dram_tensor("xr", [C, T], FP8, kind="ExternalInput").ap()
    wq_in = nc.dram_tensor("wqT", [C, OC3], FP8, kind="ExternalInput").ap()
    wr_in = (
        nc.dram_tensor("wrT", [C, 2048], FP8, kind="ExternalInput").ap()
        if USE_WR
        else None
    )
    bq_in = nc.dram_tensor("bqc", [128, 24], F32, kind="ExternalInput").ap()
    wo_in = nc.dram_tensor("woT", [C, C], FP8, kind="ExternalInput").ap()
    out_t = nc.dram_tensor("outT", [T, C], BF16, kind="ExternalOutput").ap()
    # DRAM scratch: exp(scores) as (g, (h, t_abs)) and V as (g, (d, t_abs))
    se_d = nc.dram_tensor("se_d", [16, H * T], BF16).ap()
    vt_d = nc.dram_tensor("vt_d", [16, DK * T], BF16).ap()
    sev_d = se_d.rearrange("g (h t) -> g h t", h=H)
    vtv_d = vt_d.rearrange("(m hl) (d t) -> hl d m t", m=8, hl=2, d=DK)
    vbv_d = vt_d.rearrange("g (d t) -> g d t", d=DK)

    Exp = mybir.ActivationFunctionType.Exp
    Copy = mybir.ActivationFunctionType.Copy
    Ident = mybir.ActivationFunctionType.Identity
    ADD = mybir.AluOpType.add
    MUL = mybir.AluOpType.mult
    DR = mybir.MatmulPerfMode.DoubleRow

    with tile.TileContext(nc) as tc, contextlib.ExitStack() as octx:
        const = octx.enter_context(tc.tile_pool(name="const", bufs=1))
        bqc = const.tile([128, 24], F32, tag="bqc")
        sesum = const.tile([16, H * SP1], BF16, tag="sesum")
        zmain = const.tile([16, 16], F32, tag="zmain")
        rrecf = const.tile([16, 16], F32, tag="rrecf")
        rrecs = const.tile([16, 16], BF16, tag="rrecs")
        rrec_exp = const.tile([16, H * 64], BF16, tag="rrec_exp")
        sebpool = octx.enter_context(tc.tile_pool(name="seb", bufs=4))
        sebs = {}

        def emit_sebload(si):
            seb = sebpool.tile([16, H * SS], BF16, tag="seb")
            nc.gpsimd.dma_start(
                seb[:].rearrange("p (h t) -> p h t", h=H),
                sev_d[:, :, si * SS : (si + 1) * SS],
            )
            sebs[si] = seb

        # ---------------- PASS 1 ----------------
        with contextlib.ExitStack() as ctx:
            wpool = ctx.enter_context(tc.tile_pool(name="wq", bufs=1))
            xpool = ctx.enter_context(tc.tile_pool(name="x", bufs=2))
            stpool = ctx.enter_context(tc.tile_pool(name="stage", bufs=2))
            qkpool = ctx.enter_context(tc.tile_pool(name="qkt", bufs=2))
            sepool = ctx.enter_context(tc.tile_pool(name="se", bufs=2))
            zpool = ctx.enter_context(tc.tile_pool(name="zp", bufs=2))
            ps_a = ctx.enter_context(tc.tile_pool(name="psA", bufs=6, space="PSUM"))
            ps_s = ctx.enter_context(tc.tile_pool(name="psS", bufs=2, space="PSUM"))

            xs, xrs = {}, {}

            def emit_xload(s):
                xk = xpool.tile([128, 8 * SP1], FP8, tag="x")
                nc.sync.dma_start(
                    xk[:].rearrange("p (k t) -> p k t", k=8),
                    x_in[:, s * SP1 : (s + 1) * SP1].rearrange(
                        "(k p) t -> p k t", k=8
                    ),
                )
                xs[s] = xk
                xrk = xpool.tile([128, 8 * SP1], FP8, tag="xr")
                nc.sync.dma_start(
                    xrk[:].rearrange("p (k t) -> p k t", k=8),
                    xr_in[:, s * SP1 : (s + 1) * SP1].rearrange(
                        "(k p) t -> p k t", k=8
                    ),
                )
                xrs[s] = xrk

            # Weight tiles: wqp[j] holds c-chunk pair (2j, 2j+1) as
            # (128, (2, 3072)) fp8 for DoubleRow. Span 0 is paced by these:
            # load in column-quarters interleaved with per-k x slices.
            wq_sb = []
            for j in range(4):
                w = wpool.tile([128, 2 * OC3], FP8, tag=f"wq{j}", name=f"wq{j}")
                wq_sb.append(w)
            wqv = [w[:].rearrange("p (i m) -> p i m", i=2) for w in wq_sb]
            if USE_WR:
                wr_sb = [
                    wpool.tile([128, 2 * 2048], FP8, tag=f"wr{j}", name=f"wr{j}")
                    for j in range(4)
                ]
                wrv = [w[:].rearrange("p (i m) -> p i m", i=2) for w in wr_sb]
            x0 = xpool.tile([128, 8 * SP1], FP8, tag="x", name="x0")
            xs[0] = x0
            xr0 = xpool.tile([128, 8 * SP1], FP8, tag="xr", name="xr0")
            xrs[0] = xr0
            # span 0 pacing: K columns (1024:2048) load first (K pairs run
            # first), then Q (0:1024), then V (2048:3072); x8 then xr.
            for j in range(4):
                nc.sync.dma_start(
                    x0[:, (2 * j) * SP1 : (2 * j + 1) * SP1],
                    x_in[(2 * j) * 128 : (2 * j + 1) * 128, 0:SP1],
                )
                nc.sync.dma_start(
                    x0[:, (2 * j + 1) * SP1 : (2 * j + 2) * SP1],
                    x_in[(2 * j + 1) * 128 : (2 * j + 2) * 128, 0:SP1],
                )
                nc.sync.dma_start(
                    wqv[j][:, :, 1024:2048],
                    wq_in[2 * j * 128 : (2 * j + 2) * 128, 1024:2048].rearrange(
                        "(i p) m -> p i m", i=2
                    ),
                )
                if j == 0:
                    nc.sync.dma_start(bqc[:], bq_in)
            for j in range(4):
                nc.sync.dma_start(
                    xr0[:].rearrange("p (k t) -> p k t", k=8)[:, 2 * j : 2 * j + 2, :],
                    xr_in[2 * j * 128 : (2 * j + 2) * 128, 0:SP1].rearrange(
                        "(i p) t -> p i t", i=2
                    ),
                )
                if USE_WR:
                    nc.sync.dma_start(
                        wrv[j][:, :, 1024:2048],
                        wr_in[2 * j * 128 : (2 * j + 2) * 128, 1024:2048].rearrange(
                            "(i p) m -> p i m", i=2
                        ),
                    )
            for j in range(4):
                nc.sync.dma_start(
                    wqv[j][:, :, 0:1024],
                    wq_in[2 * j * 128 : (2 * j + 2) * 128, 0:1024].rearrange(
                        "(i p) m -> p i m", i=2
                    ),
                )
                if USE_WR:
                    nc.sync.dma_start(
                        wrv[j][:, :, 0:1024],
                        wr_in[2 * j * 128 : (2 * j + 2) * 128, 0:1024].rearrange(
                            "(i p) m -> p i m", i=2
                        ),
                    )
            for j in range(4):
                nc.sync.dma_start(
                    wqv[j][:, :, 2048:3072],
                    wq_in[2 * j * 128 : (2 * j + 2) * 128, 2048:3072].rearrange(
                        "(i p) m -> p i m", i=2
                    ),
                )

            qts, kts, ses = {}, {}, {}
            # PSUM evacuations are Act/DVE only (GPSIMD cannot touch PSUM).
            # Units are 512-col halves; DVE coalesces a pair's two same-hl
            # halves into one (64,(2,512)) tensor_tensor with broadcast bias.
            picker = _EvacPicker(nc, [("act", 1.0), ("dve", 1.0)])

            def evac_qk(dvw, ps, mm, hl, m):
                """One head-half of a q/k m-tile: psum (64,512) -> qt/kt head
                2mm+hl (m's half), out = psum + 32*bias."""
                eng = picker.pick()
                dst = dvw[:, (m % 8) : (m % 8) + 1, hl, :]
                src = ps[hl * 64 : (hl + 1) * 64, :]
                b = bqc[hl * 64 : (hl + 1) * 64, m : m + 1]
                if eng == "dve":
                    nc.vector.tensor_scalar(dst, src, b, None, ADD)
                else:
                    nc.scalar.activation(dst, src, Ident, bias=b)

            def evac_v(stage, ps, m):
                eng = picker.pick()
                mm = m - 16
                dst = stage[:, mm * SP1 : (mm + 1) * SP1]
                b = bqc[:, m : m + 1]
                if eng == "dve":
                    nc.vector.tensor_scalar(dst, ps[:], b, None, ADD)
                else:
                    nc.scalar.activation(dst, ps[:], Ident, bias=b)

            NBLK1 = SP1 // 32  # 32-t scores blocks per span

            def emit_scores_block(s, blk, quarter_zred=False):
                """One 32-t scores block of span s: 32 Gram matmuls + fused
                exp(x/8192) on ScalarE into the span's se tile. Z bookkeeping
                runs per 128-t quarter: spans 0..6 fold into the bf16 sesum
                tree (with the big reduce quartered under the last span);
                the last span quarter-reduces directly into zmain."""
                if blk == 0:
                    ses[s] = sepool.tile([16, H * SP1], BF16, tag="se", name="se")
                se = ses[s]
                qtv = qts[s][:].rearrange("p (h t) -> p t h", h=H)
                ktv = kts[s][:].rearrange("p (g t) -> p t g", g=H)
                sev = se[:].rearrange("p (h t) -> p t h", h=H)
                pss = ps_s.tile([16, 512], F32, tag="psS")
                for s32 in range(32):
                    tl = blk * 32 + s32
                    nc.tensor.matmul(
                        pss[:, s32 * 16 : (s32 + 1) * 16],
                        lhsT=ktv[:, tl, :],
                        rhs=qtv[:, tl, :],
                        start=True,
                        stop=True,
                    )
                nc.scalar.activation(
                    sev[:, blk * 32 : (blk + 1) * 32, :],
                    pss[:].rearrange("p (t h) -> p t h", h=H),
                    Exp,
                    scale=1.0 / 8192.0,
                )
                if blk % 4 == 3:
                    q = blk // 4
                    if quarter_zred:
                        zp = zpool.tile([16, 16], F32, tag="zp")
                        nc.vector.tensor_reduce(
                            zp[:],
                            se[:].rearrange("p (h t) -> p h t", h=H)[
                                :, :, q * 128 : (q + 1) * 128
                            ],
                            axis=mybir.AxisListType.X,
                            op=ADD,
                        )
                        nc.vector.tensor_tensor(
                            out=zmain[:], in0=zmain[:], in1=zp[:], op=ADD
                        )
                    else:
                        emit_zacc_q(s, q, se)
                        if s == NSP1 - 2:
                            # all spans 0..6 are in this sesum quarter now:
                            # the big reduce runs quartered, well before the
                            # last span's tail.
                            sq = sesum[:].rearrange(
                                "p (h f t) -> p h f t", h=H, f=4
                            )[:, :, q, :]
                            if q == 0:
                                nc.vector.tensor_reduce(
                                    zmain[:], sq, axis=mybir.AxisListType.X, op=ADD
                                )
                            else:
                                zp = zpool.tile([16, 16], F32, tag="zp")
                                nc.vector.tensor_reduce(
                                    zp[:], sq, axis=mybir.AxisListType.X, op=ADD
                                )
                                nc.vector.tensor_tensor(
                                    out=zmain[:], in0=zmain[:], in1=zp[:], op=ADD
                                )
                if blk == NBLK1 - 1:
                    qts.pop(s)
                    kts.pop(s)
                    nc.sync.dma_start(
                        sev_d[:, :, s * SP1 : (s + 1) * SP1],
                        se[:].rearrange("p (h t) -> p h t", h=H),
                    )
                    if not quarter_zred:
                        ses.pop(s)

            def emit_zacc_q(s, q, se):
                """Fold quarter q of span s's se into the bf16 sesum tree."""
                sv = se[:].rearrange("p (h f t) -> p h f t", h=H, f=4)[:, :, q, :]
                dv = sesum[:].rearrange("p (h f t) -> p h f t", h=H, f=4)[:, :, q, :]
                if s == 0:
                    nc.vector.tensor_copy(dv, sv)
                else:
                    nc.vector.tensor_tensor(out=dv, in0=dv, in1=sv, op=ADD)

            def emit_qkv(s, sc=None, self_sc=False):
                """QKV projection of span s in fp8 DoubleRow, two m-tiles per
                (128,1024) PSUM pair; scores blocks of span sc interleave over
                the k/q pairs; with self_sc the span's own scores blocks (with
                quarter Z-reduces) interleave over the v pairs."""
                xall = xs.pop(s)
                xv = xall[:].rearrange("p (k t) -> p k t", k=8)
                xrall = xrs.pop(s)
                xrv = xrall[:].rearrange("p (k t) -> p k t", k=8)
                qt = qkpool.tile([64, H * SP1], BF16, tag="qt")
                kt = qkpool.tile([64, H * SP1], BF16, tag="kt")
                stage = stpool.tile([128, 8 * SP1], BF16, tag="st")
                qts[s], kts[s] = qt, kt
                qvw = qt[:].rearrange("p (a b t) -> p a b t", a=8, b=2)
                kvw = kt[:].rearrange("p (a b t) -> p a b t", a=8, b=2)
                sblk = [0, 0]  # consumed blocks: [sc, self]

                def scores_upto(n, which, span):
                    while sblk[which] < min(n, NBLK1):
                        emit_scores_block(
                            span, sblk[which], quarter_zred=(which == 1)
                        )
                        sblk[which] += 1

                MK = [(1, mk) for mk in range(8)] + [(0, mk) for mk in range(8)] + [
                    (2, mk) for mk in range(8)
                ]
                for pi, (kind, mk) in enumerate(MK):
                    m = kind * 8 + mk
                    ps = ps_a.tile([128, 512], F32, tag="psA")
                    wr_here = USE_WR and kind < 2
                    for j in range(4):
                        nc.tensor.matmul(
                            ps[:],
                            lhsT=wqv[j][:, :, m * 128 : (m + 1) * 128],
                            rhs=xv[:, 2 * j : 2 * j + 2, :],
                            start=(j == 0),
                            stop=False,
                            perf_mode=DR,
                        )
                    for j in range(4):
                        nc.tensor.matmul(
                            ps[:],
                            lhsT=wqv[j][:, :, m * 128 : (m + 1) * 128],
                            rhs=xrv[:, 2 * j : 2 * j + 2, :],
                            start=False,
                            stop=(not wr_here and j == 3),
                            perf_mode=DR,
                        )
                    if wr_here:
                        for j in range(4):
                            nc.tensor.matmul(
                                ps[:],
                                lhsT=wrv[j][:, :, m * 128 : (m + 1) * 128],
                                rhs=xv[:, 2 * j : 2 * j + 2, :],
                                start=False,
                                stop=(j == 3),
                                perf_mode=DR,
                            )
                    if kind < 2:
                        dvw = qvw if kind == 0 else kvw
                        for hl in range(2):
                            evac_qk(dvw, ps, mk, hl, m)
                    else:
                        evac_v(stage, ps, m)
                    if sc is not None and pi < 16:
                        scores_upto((16 * (pi + 1) + 15) // 16, 0, sc)
                    if pi == 15 and sc is not None:
                        scores_upto(NBLK1, 0, sc)
                    if self_sc and pi >= 16:
                        scores_upto(2 * (pi - 15), 1, s)
                if self_sc:
                    scores_upto(NBLK1, 1, s)
                    if s in ses:
                        ses.pop(s)
                # V spill: SBUF (hl*64+d, (m,t)) -> DRAM (g=2m+hl, (d, t_abs))
                for hl in range(2):
                    nc.sync.dma_start(
                        vtv_d[hl, :, :, s * SP1 : (s + 1) * SP1],
                        stage[hl * 64 : (hl + 1) * 64, :].rearrange(
                            "d (m t) -> d m t", m=8
                        ),
                    )

            for s in range(NSP1):
                last = s == NSP1 - 1
                emit_qkv(s, sc=s - 1 if s >= 1 else None, self_sc=last)
                if s + 1 < NSP1:
                    emit_xload(s + 1)
                if s == NSP1 - 2:
                    for si0 in range(4):
                        emit_sebload(si0)
            nc.vector.reciprocal(rrecf[:], zmain[:])
            nc.vector.tensor_scalar(rrecs[:], rrecf[:], S_A, None, MUL)
            # pre-broadcast (16,16) -> (16,(16h,64)) so pass-2 norm runs 2x
            nc.vector.tensor_copy(
                rrec_exp[:].rearrange("p (h t) -> p h t", h=H),
                rrecs[:].unsqueeze(2).broadcast_to([16, 16, 64]),
            )

        # ---------------- PASS 2 ----------------
        with contextlib.ExitStack() as ctx:
            wopool = ctx.enter_context(tc.tile_pool(name="wo", bufs=1))
            wo_sb = []
            for u in range(4):
                w = wopool.tile([128, 2 * C], FP8, tag=f"wo{u}", name=f"wo{u}")
                nc.sync.dma_start(
                    w[:].rearrange("p (i o) -> p i o", i=2),
                    wo_in[2 * u * 128 : (2 * u + 2) * 128, :].rearrange(
                        "(i p) o -> p i o", i=2
                    ),
                )
                wo_sb.append(w)
            wov = [w[:].rearrange("p (i o) -> p i o", i=2) for w in wo_sb]

            vtpool = ctx.enter_context(tc.tile_pool(name="vt2", bufs=3))
            cnpool = ctx.enter_context(tc.tile_pool(name="cnat", bufs=3))
            opool = ctx.enter_context(tc.tile_pool(name="osb", bufs=3))
            ps_c = ctx.enter_context(tc.tile_pool(name="psC", bufs=4, space="PSUM"))
            ps_o = ctx.enter_context(tc.tile_pool(name="psO", bufs=4, space="PSUM"))

            OUT_SCALE = 1.0 / (W_SCALE * W_SCALE * S_A)
            p2pick = _EvacPicker(nc, [("act", 6.0), ("dve", 2.0)])

            def evac2(dst, src, scale=None):
                eng = p2pick.pick()
                if scale is None:
                    if eng == "act":
                        nc.scalar.activation(dst, src, Copy)
                    else:
                        nc.vector.tensor_copy(dst, src)
                else:
                    if eng == "act":
                        nc.scalar.activation(dst, src, Copy, scale=scale)
                    else:
                        nc.vector.tensor_scalar(dst, src, scale, None, MUL)

            norm = {}

            def emit_norm(si):
                """Normalize exp(S) for sub-span si in place (DVE 2x),
                loading it first unless prefetched during pass 1."""
                if si in sebs:
                    seb = sebs.pop(si)
                else:
                    emit_sebload(si)
                    seb = sebs.pop(si)
                sv = seb[:].rearrange("p (h f q) -> p h f q", h=H, q=64)
                rv = (
                    rrec_exp[:]
                    .rearrange("p (h q) -> p h q", h=H)
                    .unsqueeze(2)
                    .broadcast_to([16, H, 1, 64])
                )
                for f in range(SS // 64):
                    nc.vector.tensor_tensor(
                        out=sv[:, :, f : f + 1, :],
                        in0=sv[:, :, f : f + 1, :],
                        in1=rv,
                        op=MUL,
                    )
                norm[si] = seb

            vts = {}

            def emit_vtload(si):
                vt = vtpool.tile([16, DK * SS], BF16, tag="vt2")
                nc.gpsimd.dma_start(
                    vt[:].rearrange("p (d t) -> p d t", d=DK),
                    vbv_d[:, :, si * SS : (si + 1) * SS],
                )
                vts[si] = vt

            cnats = {}

            def emit_ctx(si, op_si=None):
                """Context matmuls for sub-span si, PSUM in (c-chunk, t)
                layout: per t two (64d x 8he) matmuls at partition 64*hl,
                he strided 64 cols; per 64-t bank one evac -> fp8 cnat."""
                cn = cnpool.tile([128, 8 * SS], FP8, tag="cnat", name="cnat")
                cnats[si] = cn
                cnv = cn[:].rearrange("p (u t) -> p u t", u=8)
                atv = norm.pop(si)[:].rearrange(
                    "p (he hl t) -> p he hl t", he=8, hl=2
                )
                vtv = vts.pop(si)[:].rearrange("p (d t) -> p d t", d=DK)
                for q4 in range(4):  # 64-t quarters
                    psc = ps_c.tile([128, 512], F32, tag="psC")
                    pscv = psc[:].rearrange("p (u t) -> p u t", u=8)
                    for tq in range(64):
                        tl = q4 * 64 + tq
                        for hl in range(2):
                            nc.tensor.matmul(
                                pscv[64 * hl : 64 * hl + 64, :, tq],
                                lhsT=vtv[:, :, tl],
                                rhs=atv[:, :, hl, tl],
                                start=True,
                                stop=True,
                            )
                    evac2(cnv[:, :, q4 * 64 : (q4 + 1) * 64], pscv)
                    # keep the PE dense: big DR out-projection matmuls of the
                    # previous sub-span slot in between the tiny ctx matmuls
                    if op_si is not None and q4 % 2 == 0:
                        emit_outproj_mt(op_si, q4 // 2)

            def emit_outproj_mt(si, mt):
                """fp8 DoubleRow out-projection of one 128-t tile from cnat."""
                cn = cnats[si]
                cnv = cn[:].rearrange("p (u t) -> p u t", u=8)
                tB0 = si * SS
                osb = opool.tile([128, C], BF16, tag="osb", name="osb")
                for n in range(2):
                    pso = ps_o.tile([128, 512], F32, tag="psO")
                    for u in range(4):
                        nc.tensor.matmul(
                            pso[:],
                            lhsT=cnv[:, 2 * u : 2 * u + 2, mt * 128 : (mt + 1) * 128],
                            rhs=wov[u][:, :, n * 512 : (n + 1) * 512],
                            start=(u == 0),
                            stop=(u == 3),
                            perf_mode=DR,
                        )
                    evac2(osb[:, n * 512 : (n + 1) * 512], pso[:], OUT_SCALE)
                nc.sync.dma_start(
                    out_t[tB0 + mt * 128 : tB0 + mt * 128 + 128, :], osb[:]
                )
                if mt == SS // 128 - 1:
                    cnats.pop(si)

            emit_norm(0)
            emit_norm(1)
            emit_norm(2)
            emit_vtload(0)
            emit_vtload(1)
            for si in range(NSS):
                if si + 3 < NSS:
                    emit_norm(si + 3)
                if 2 <= si + 2 < NSS:
                    emit_vtload(si + 2)
                emit_ctx(si, op_si=si - 1 if si >= 1 else None)
            emit_outproj_mt(NSS - 1, 0)
            emit_outproj_mt(NSS - 1, 1)

    # NOTE: _dedup_ldweights(nc) breaks hardware execution (walrus pairs
    # LDWEIGHTS/MATMULT positionally) and saves ~nothing in the cost model.
    _split_sync_waits(nc, limit=1)
    return nc


_NC_CACHE = {}


def _get_nc(T, SPAN):
    key = (T, SPAN)
    if key not in _NC_CACHE:
        _NC_CACHE[key] = build_kernel(T, SPAN)
    return _NC_CACHE[key]


def _prep_weights(w_qkv, b_qkv, w_out):
    f8 = ml_dtypes.float8_e4m3
    w3 = w_qkv.reshape(H, 192, C).astype(np.float32)
    qw = w3[:, :DK, :].reshape(H * DK, C)
    kw = w3[:, DK : 2 * DK, :].reshape(H * DK, C)
    vw = w3[:, 2 * DK :, :].reshape(H * DK, C)
    wall = np.concatenate([qw, kw, vw], axis=0) * W_SCALE  # (3072, C)
    w8 = wall.astype(f8)
    wqT = w8.T.copy()  # (C, 3072) fp8
    # weight residual (q/k rows only): 32*(w - w8/32) in fp8
    wrT = (wall[:2048] - w8[:2048].astype(np.float32)).astype(f8).T.copy()
    b3 = b_qkv.reshape(H, 192).astype(np.float32)
    bq = np.concatenate(
        [
            b3[:, :DK].reshape(-1),
            b3[:, DK : 2 * DK].reshape(-1),
            b3[:, 2 * DK :].reshape(-1),
        ]
    )
    # biases are added to the x32-scaled PSUM, so pre-scale them too
    bqc = np.ascontiguousarray(bq.reshape(24, 128).T * W_SCALE).astype(
        np.float32
    )  # (128, 24)
    woT = (w_out.T.astype(np.float32) * W_SCALE).copy().astype(f8)  # (C, C)
    return wqT, wrT, bqc, woT


def kernel(x, w_qkv, b_qkv, w_out, b_out, _trace=False, _span=256):
    x = np.asarray(x)
    w_qkv = np.asarray(w_qkv)
    b_qkv = np.asarray(b_qkv)
    w_out = np.asarray(w_out)
    b_out = np.asarray(b_out)
    B, _, T = x.shape
    assert B == N_CORES
    nc = _get_nc(T, _span)
    wqT, wrT, bqc, woT = _prep_weights(w_qkv, b_qkv, w_out)
    f8 = ml_dtypes.float8_e4m3
    in_maps = []
    for b in range(B):
        x8 = x[b].astype(f8)
        xr = (x[b].astype(np.float32) - x8.astype(np.float32)).astype(f8)
        im = {
            "x": x8,
            "xr": xr,
            "wqT": wqT,
            "bqc": bqc,
            "woT": woT,
        }
        if USE_WR:
            im["wrT"] = wrT
        in_maps.append(im)
    res = run_bass_kernel_spmd(nc, in_maps, list(range(N_CORES)), trace=_trace)
    out = np.stack(
        [np.asarray(res.results[b]["outT"]).astype(np.float32).T for b in range(B)],
        axis=0,
    )
    out += b_out.astype(np.float32)[None, :, None]
    if _trace:
        kernel.last_exec_time_ns = res.exec_time_ns
        kernel.last_results = res
    return out
